# revision 4
# baseline (speedup 1.0000x reference)
"""Trainium2 Bass kernel for multi-head self-attention (nn_CrossAttention).

Reference computation (B=2, S=4096, C=512, H=8 heads, Dh=64):
    q = hid @ Wq.T; k = hid @ Wk.T; v = hid @ Wv.T     (per-head split)
    out = softmax(q k^T / sqrt(Dh)) v                   (per head)
    final = concat_heads(out) @ Wo.T + bo

Sharding: batch*head parallel. 16 (batch, head) units over 8 cores ->
each core owns one batch b and two adjacent heads. Each core computes a
*partial* output projection (its two heads' contribution to final[b]);
the host sums 4 partials per batch and adds the bias.

Device-side layout choices (see comments inline):
  - hidden is passed pre-transposed (hidT [C, S]) so the model dim (the
    contraction dim of all projections) lies on SBUF partitions.
  - q, k are kept transposed on-chip: qT/kT [128=2*Dh, S].
  - scores are computed transposed (sT [kv, q]) so the PV matmul needs no
    transposition of the 16M-element probability matrix; softmax needs no
    max-pass (scores are O(1) by construction) and the denominator comes
    free from an appended ones-column in V.
  - all matmuls run as float32r (FP22 truncation, full PE rate).
"""

import numpy as np

import concourse.bacc as bacc
import concourse.bass as bass
import concourse.tile as tile
from concourse import mybir
from concourse.bass_utils import run_bass_kernel_spmd

B, S, C = 2, 4096, 512
H, DH = 8, 64
HL = 2                # heads per core
DL = HL * DH          # 128, local projection width
N_CORES = 8
CC = C // 128         # 4 contraction chunks for projections
NQ = S // 512         # 8 q-chunks of 512
NJ = S // 128         # 32 kv-chunks of 128
QG = 1024             # exp granularity along q
NQG = S // QG

F32 = mybir.dt.float32
F32R = mybir.dt.float32r
EXP = mybir.ActivationFunctionType.Exp
RECIP = mybir.ActivationFunctionType.Reciprocal


def _emit(tc, nc, hidT, wqT, wkT, wvT, woT, outp):
    with tc.tile_pool(name="persist", bufs=1) as persist:
        qT = persist.tile([DL, S], F32R)
        kT = persist.tile([DL, S], F32R)
        v0 = persist.tile([128, NJ, DH + 1], F32R)   # V plus ones col, head 0
        v1 = persist.tile([128, NJ, DH + 1], F32R)   # head 1
        wo0 = persist.tile([DH, C], F32R)
        wo1 = persist.tile([DH, C], F32R)
        oT0 = persist.tile([DH + 1, S], F32)        # out^T accum + rowsum row
        oT1 = persist.tile([DH + 1, S], F32)

        # whole-tile memset to 1.0: data cols 0..63 are overwritten by the V
        # projection copies below; col 64 stays 1.0 (the rowsum ones-column)
        nc.gpsimd.memset(v0[:].bitcast(F32), 1.0)
        nc.gpsimd.memset(v1[:].bitcast(F32), 1.0)
        nc.gpsimd.memset(oT0[:], 0.0)
        nc.gpsimd.memset(oT1[:], 0.0)

        nc.sync.dma_start(out=wo0[:], in_=woT[0:DH, :])
        nc.sync.dma_start(out=wo1[:], in_=woT[DH:DL, :])

        # ---- phase A: load hidT + weights, project q/k (transposed) and v ----
        with tc.tile_pool(name="hload", bufs=1) as hload, \
             tc.tile_pool(name="wload", bufs=1) as wload, \
             tc.tile_pool(name="pjq", bufs=4, space="PSUM") as pjq, \
             tc.tile_pool(name="pjv", bufs=4, space="PSUM") as pjv:
            hid_sb = hload.tile([128, CC, S], F32R)
            hidT_r = hidT.rearrange("(cc p) s -> p cc s", p=128)
            for cc in range(CC):
                nc.sync.dma_start(out=hid_sb[:, cc, :], in_=hidT_r[:, cc, :])

            wq_sb = wload.tile([128, CC, DL], F32R)
            wk_sb = wload.tile([128, CC, DL], F32R)
            wv_sb = wload.tile([128, CC, DL], F32R)
            for w_sb, w_dram in ((wq_sb, wqT), (wk_sb, wkT), (wv_sb, wvT)):
                nc.sync.dma_start(
                    out=w_sb[:], in_=w_dram.rearrange("(cc p) d -> p cc d", p=128)
                )

            # qT/kT: psum[m,n] = sum_c W[m,c] hid[n,c] = qT[dl, s]
            for dst, w_sb in ((qT, wq_sb), (kT, wk_sb)):
                for sc in range(NQ):
                    ps = pjq.tile([DL, 512], F32)
                    for cc in range(CC):
                        nc.tensor.matmul(
                            ps[:],
                            lhsT=w_sb[:, cc, :],
                            rhs=hid_sb[:, cc, sc * 512:(sc + 1) * 512],
                            start=(cc == 0),
                            stop=(cc == CC - 1),
                        )
                    nc.vector.tensor_copy(dst[:, sc * 512:(sc + 1) * 512], ps[:].bitcast(F32R))

            # v natural: psum[m,n] = sum_c hid[m,c] Wv[n,c] = v[s, dl]
            for jc in range(NJ):
                ps = pjv.tile([128, DL], F32)
                for cc in range(CC):
                    nc.tensor.matmul(
                        ps[:],
                        lhsT=hid_sb[:, cc, jc * 128:(jc + 1) * 128],
                        rhs=wv_sb[:, cc, :],
                        start=(cc == 0),
                        stop=(cc == CC - 1),
                    )
                nc.vector.tensor_copy(v0[:, jc, 0:DH], ps[:, 0:DH].bitcast(F32R))
                nc.vector.tensor_copy(v1[:, jc, 0:DH], ps[:, DH:DL].bitcast(F32R))

        # ---- phase B: attention, kv-chunk outer loop ----
        with tc.tile_pool(name="scps", bufs=2, space="PSUM") as scps, \
             tc.tile_pool(name="pvps", bufs=4, space="PSUM") as pvps, \
             tc.tile_pool(name="ptsb", bufs=3) as ptsb:
            for jc in range(NJ):
                for h, (oT, vh) in enumerate(((oT0, v0), (oT1, v1))):
                    hp = h * DH
                    for qg in range(NQG):
                        # scores^T [kv=128, q=QG] via K=64 matmul in row-group hp
                        st = scps.tile([128, QG], F32)
                        for q2 in range(QG // 512):
                            qo = qg * QG + q2 * 512
                            nc.tensor.matmul(
                                st[:, q2 * 512:(q2 + 1) * 512],
                                lhsT=kT[hp:hp + DH, jc * 128:(jc + 1) * 128],
                                rhs=qT[hp:hp + DH, qo:qo + 512],
                                start=True,
                                stop=True,
                            )
                        # exp(score/8), no max subtraction (scores are O(1))
                        pt = ptsb.tile([128, QG], F32R)
                        nc.scalar.activation(pt[:], st[:], EXP, scale=0.125)
                        # outT[d, q] += sum_j V[j, d] * P[j, q]; row 64 = rowsum
                        for q2 in range(QG // 512):
                            qo = qg * QG + q2 * 512
                            pv = pvps.tile([DH + 1, 512], F32)
                            nc.tensor.matmul(
                                pv[:],
                                lhsT=vh[:, jc, :],
                                rhs=pt[:, q2 * 512:(q2 + 1) * 512],
                                start=True,
                                stop=True,
                            )
                            nc.vector.tensor_add(
                                oT[:, qo:qo + 512], oT[:, qo:qo + 512], pv[:]
                            )

        # ---- phase C: normalize by rowsum, partial output projection ----
        with tc.tile_pool(name="norm", bufs=2) as norm, \
             tc.tile_pool(name="ndram", bufs=2, space="DRAM") as ndram, \
             tc.tile_pool(name="opps", bufs=2, space="PSUM") as opps, \
             tc.tile_pool(name="otsb", bufs=3) as otsb:
            oTn = []
            for h, oT in enumerate((oT0, oT1)):
                # rowsum row -> DRAM -> [128, S/128] reshape so the (slow,
                # per-lane) DVE reciprocal runs on all 128 lanes
                srow = ndram.tile([1, S], F32)
                nc.sync.dma_start(out=srow[:], in_=oT[DH:DH + 1, :])
                rs = norm.tile([128, S // 128], F32)
                nc.sync.dma_start(
                    out=rs[:], in_=srow[0, :].rearrange("(p f) -> p f", p=128)
                )
                nc.vector.reciprocal(rs[:], rs[:])
                rrow = ndram.tile([1, S], F32)
                nc.sync.dma_start(
                    out=rrow[0, :].rearrange("(p f) -> p f", p=128), in_=rs[:]
                )
                rb = norm.tile([DH, S], F32)
                r0 = rrow[0, :]
                bcast = bass.AP(tensor=r0.tensor, offset=r0.offset,
                                ap=[[0, DH]] + list(r0.ap))
                nc.sync.dma_start(out=rb[:], in_=bcast)
                on = norm.tile([DH, S], F32R, tag=f"oTn{h}")
                nc.vector.tensor_mul(on[:], oT[0:DH, :].bitcast(F32R),
                                     rb[:].bitcast(F32R))
                oTn.append(on)

            for sc in range(S // 128):
                po = opps.tile([128, C], F32)
                nc.tensor.matmul(
                    po[:],
                    lhsT=oTn[0][:, sc * 128:(sc + 1) * 128],
                    rhs=wo0[:],
                    start=True,
                    stop=False,
                )
                nc.tensor.matmul(
                    po[:],
                    lhsT=oTn[1][:, sc * 128:(sc + 1) * 128],
                    rhs=wo1[:],
                    start=False,
                    stop=True,
                )
                ot = otsb.tile([128, C], F32)
                nc.vector.tensor_copy(ot[:], po[:])
                nc.sync.dma_start(out=outp[sc * 128:(sc + 1) * 128, :], in_=ot[:])


def build_nc():
    nc = bacc.Bacc("TRN2", target_bir_lowering=False, debug=False)
    hidT = nc.dram_tensor("hidT", [C, S], F32R, kind="ExternalInput").ap()
    wqT = nc.dram_tensor("wqT", [C, DL], F32R, kind="ExternalInput").ap()
    wkT = nc.dram_tensor("wkT", [C, DL], F32R, kind="ExternalInput").ap()
    wvT = nc.dram_tensor("wvT", [C, DL], F32R, kind="ExternalInput").ap()
    woT = nc.dram_tensor("woT", [DL, C], F32R, kind="ExternalInput").ap()
    outp = nc.dram_tensor("outp", [S, C], F32, kind="ExternalOutput").ap()
    with tile.TileContext(nc) as tc:
        _emit(tc, nc, hidT, wqT, wkT, wvT, woT, outp)
    nc.compile()
    return nc


def make_in_maps(hidden_states, Wq, Wk, Wv, Wo):
    """Shard the full inputs into 8 per-core input maps."""
    hs = np.asarray(hidden_states, dtype=np.float32)
    hidT_b = [np.ascontiguousarray(hs[b].T) for b in range(B)]
    in_maps = []
    for core in range(N_CORES):
        b = core // 4
        p = core % 4
        lo, hi = 2 * p * DH, (2 * p + 2) * DH
        in_maps.append({
            "hidT": hidT_b[b],
            "wqT": np.ascontiguousarray(np.asarray(Wq, np.float32)[lo:hi, :].T),
            "wkT": np.ascontiguousarray(np.asarray(Wk, np.float32)[lo:hi, :].T),
            "wvT": np.ascontiguousarray(np.asarray(Wv, np.float32)[lo:hi, :].T),
            "woT": np.ascontiguousarray(np.asarray(Wo, np.float32)[:, lo:hi].T),
        })
    return in_maps


def gather_output(results, bo):
    """Sum the 4 per-core partial projections per batch, add bias."""
    bo = np.asarray(bo, np.float32)
    out = np.empty((B, S, C), np.float32)
    for b in range(B):
        acc = results[4 * b]["outp"].astype(np.float32).copy()
        for p in range(1, 4):
            acc += results[4 * b + p]["outp"]
        out[b] = acc + bo
    return out


_NC_CACHE = None


def _get_nc():
    global _NC_CACHE
    if _NC_CACHE is None:
        _NC_CACHE = build_nc()
    return _NC_CACHE


def kernel(hidden_states, Wq, Wk, Wv, Wo, bo, _trace=False, _res_out=None):
    nc = _get_nc()
    in_maps = make_in_maps(hidden_states, Wq, Wk, Wv, Wo)
    res = run_bass_kernel_spmd(nc, in_maps, list(range(N_CORES)), trace=_trace)
    if _res_out is not None:
        _res_out.append(res)
    return gather_output(res.results, bo)
